# revision 29
# baseline (speedup 1.0000x reference)
"""Trainium2 Bass kernel for nn_DotAtt_40097814675537.

Math (matches the reference exactly up to fp rounding):
    score = Q @ K^T / sqrt(d)        [B, Sq, Sk]
    x     = score @ V                [B, Sq, dv]
    out   = softmax(where(j > valid_len[q], -1e6, x[b, q, j]), axis=-1)

Optimizations:
  * Associativity: x = (Q / sqrt(d)) @ (K^T @ V) - 4x fewer FLOPs.
  * Data-parallel over batch B=8, one batch per NeuronCore, no collectives.
  * Single-pass fp16 matmuls (rel err 2.7e-3 measured, 7x margin).
  * Sorted-query specialization: host sorts queries by valid_len (row
    permutation is exact for row-wise softmax); each 128-row tile only
    computes columns [0, tile max valid_len + 1).  Host inverse-permutes
    and normalizes (divide by row sum) in fp32.
  * STRIP masks: after sorting, rows of a tile have nearly equal
    valid_len, so the additive -60000 mask is nonzero only on a narrow
    column strip [lo, W) per tile (lo = floor32(min_vl+1)).  Only the
    strip is loaded from DRAM (~0.2 MB vs 1.18 MB full mask), and it is
    accumulated into the PSUM x tile BY THE PE - an extra matmul with a
    128x128 identity as the stationary operand inside the accumulation
    group (out += I @ strip, exact in fp16) - so the DVE does only the
    max-reduce.  The reduce and the exp read PSUM directly (no SBUF xs
    copy); DVE phase-2 work drops ~2.5x vs add-full + reduce-full.
  * ~36 128-wide warm-up matmuls on a zeroed tile bridge the HAM clock
    gate (PE starts at 1.2 GHz, needs ~3.4us of sustained activity to
    reach 2.4 GHz) across the first KV chunk's DMA completion latency
    (~3.2us), so all 64 phase-1 matmuls run at full clock.
  * Phase-1 tail is c-major so each M psum bank stops several matmuls
    before phase-1 ends; casts to fp16 (c0,c1,c2 on DVE, c3 on ACT)
    overlap the tail and phase 2 starts without a bubble.
  * Tiles processed widest-first and PAIRED (pair width = max of two)
    so two tiles share one output store; unnormalized exp(x-max) is
    stored in fp16 (half the output bytes).
  * All loads/stores on the Sync HWDGE ring in consumption order.
    (Scalar/Activation HWDGE ring crashes the exec unit; GpSimd SWDGE
    steals HBM bandwidth from K/V during the phase-1 ramp.
    tensor_mask_reduce / tensor_tensor_reduce would fuse the mask+max
    into one DVE op but both crash the DVE on this runtime.)
"""

import math
import sys
import types

import numpy as np

B, SQ, SK, D, DV = 8, 2048, 2048, 512, 512
N_CORES = 8
P = 128  # partitions
SC = SK // P  # 16 s-chunks for the K^T V contraction
DC = D // P  # 4 d-chunks for the Q M contraction
QT_TILES = SQ // P  # 16 query row tiles
NPAIR = QT_TILES // 2
NEG_FILL = -60000.0  # fits f16; exp() still underflows to exactly 0
N_WARM = 31  # 128-wide warm-up matmuls (~3.3us cold)

_CACHE = {}


def _install_ntff_hook():
    """antenv.axon_hooks is absent in this image; provide it so trace=True
    profiling works when requested (used by test.py, harmless otherwise)."""
    if "antenv.axon_hooks" in sys.modules:
        return
    try:
        from trn_agent_boot.trn_boot import _ntff_profile_via_ctypes

        hook = _ntff_profile_via_ctypes("/opt/axon/libaxon_pjrt.so")
    except Exception:
        hook = None
    mod = types.ModuleType("antenv.axon_hooks")
    mod.get_axon_ntff_profile_hook = lambda: hook
    mod.set_axon_ntff_profile_hook = lambda h: None
    sys.modules["antenv.axon_hooks"] = mod


def _build(pw, ss, whs):
    """pw: 8 pair widths, descending.  ss: 16 strip widths (per half).
    whs: 16 per-half reduce/exp widths (ceil32(max_vl+1), <= pair width:
    columns beyond are pure -60000 from the strip matmul, so they can
    never be the row max and their exp is exactly 0 - a GpSimd memset
    of the ex gap replaces DVE/ACT work there)."""
    import concourse.tile as tile
    from concourse import bacc, mybir

    nc = bacc.Bacc("TRN2", target_bir_lowering=False, debug=False, num_devices=N_CORES)
    f32 = mybir.dt.float32
    f16 = mybir.dt.float16

    sum_s = sum(ss)
    soffs = [0]
    for s in ss:
        soffs.append(soffs[-1] + s)

    # Layouts (partition-major):
    #   kv: [128, SC*1024] f16  kv[p, s*1024 + j]     = K[s*128+p, j] (j<512)
    #                           kv[p, s*1024 + 512+j] = V[s*128+p, j]
    #   qm: [128, 128 + sum_s + SQ*DC] f16; a 128x128 identity (stationary
    #       operand of the strip-accumulate matmuls), strip masks packed
    #       per half in consumption order, then Q^T tiles in consumption
    #       order: qm[p, hdr + i*512 + c*128 + r] = Qhat[tile_i*128+r, c*128+p]
    #   o:  [8, 128, 2, DV] f16; o[i, p, h, w] = pair i, tile-half h, row p
    KVCOLS = SC * 2 * DV
    QCOLS = QT_TILES * DC * P
    HDR = P + sum_s  # identity + strips
    kv_d = nc.dram_tensor("kv", [P, KVCOLS], f16, kind="ExternalInput")
    qm_d = nc.dram_tensor("qm", [P, HDR + QCOLS], f16, kind="ExternalInput")
    o_d = nc.dram_tensor("o", [NPAIR, P, 2, DV], f16, kind="ExternalOutput")

    CHUNK = 2 * DV  # kv columns per s-chunk
    QPB = 2 * DC * P  # qm columns per pair

    # PRE-CONTEXT (parent bb, before the TileContext entry barrier):
    # ONLY s-chunk 0's load and the PE warm-up are hoisted here - they
    # start ~1.2us earlier than anything inside the context could, which
    # pulls the first real matmul (gated by chunk 0's ~3.3us DMA
    # completion latency) forward by the same amount.  Chunks 1-15 stay
    # Tile-tracked: hoisting them all starves the Tile scheduler's timing
    # model, and it then reorders phase 2 into a c-outer order that costs
    # ~4us (and can reorder phase-1 matmuls past their manual gates).
    # The warm-up matmuls bridge the PE HAM clock gate (1.2 GHz until
    # ~3.4us of sustained activity); they read whatever garbage is in
    # SBUF - their PSUM target is reset by phase 1's start=True.
    kv0 = nc.alloc_sbuf_tensor("kv0_sb", [P, CHUNK], f16)
    warm = nc.alloc_sbuf_tensor("warm_sb", [P, P], f16)
    # warm_ps ALIASES the first pool PSUM bank (bump pointer restored after
    # the alloc): all warm-up matmuls retire before any in-context PSUM
    # write, and phase 1's start=True resets the bank's garbage.
    _psum_saved = nc.psum_base
    warm_ps = nc.alloc_psum_tensor("warm_ps", [P, P], f32)
    nc.psum_base = _psum_saved
    kv_sem = nc.alloc_semaphore("kv_sem")
    nc.sync.dma_start(out=kv0[:, :], in_=kv_d[:, 0:CHUNK]).then_inc(kv_sem, 16)
    for w in range(N_WARM):
        nc.tensor.matmul(warm_ps[:, :], warm[:, :], warm[:, :], start=True, stop=True)

    with tile.TileContext(nc) as tc:
        with (
            tc.tile_pool(name="big", bufs=1) as big,
            tc.tile_pool(name="mprime", bufs=1) as mp_pool,
            tc.tile_pool(name="psm", bufs=1, space="PSUM") as psum_m,
            tc.tile_pool(name="psx", bufs=4, space="PSUM") as psum_x,
            tc.tile_pool(name="expo", bufs=4) as expo,
            tc.tile_pool(name="stats", bufs=8) as stats,
        ):
            kvt = big.tile([P, KVCOLS - CHUNK], f16, tag="kv", name="kv_sb")
            qmt = big.tile([P, HDR + QCOLS], f16, tag="qm", name="qm_sb")

            # K/V chunks 1-15 (per-chunk loads gate phase-1 matmuls
            # finely), then identity+strips, then per-pair Q blocks, all
            # on the Sync HWDGE ring in consumption order.
            for s in range(1, SC):
                lo, hi = (s - 1) * CHUNK, s * CHUNK
                nc.sync.dma_start(out=kvt[:, lo:hi], in_=kv_d[:, lo + CHUNK : hi + CHUNK])
            nc.sync.dma_start(out=qmt[:, 0:HDR], in_=qm_d[:, 0:HDR])
            for i in range(NPAIR):
                lo, hi = HDR + i * QPB, HDR + (i + 1) * QPB
                nc.sync.dma_start(out=qmt[:, lo:hi], in_=qm_d[:, lo:hi])

            psums = [
                psum_m.tile([P, DV], f32, tag=f"m{c}", name=f"psum_m{c}")
                for c in range(DC)
            ]

            # Phase 1: M = K^T V over 16 s-chunks, single fp16 pass.
            # Chunk 0 reads the raw kv0 (outside Tile's dependency
            # tracking): ALL four of its matmuls get a kv_sem wait
            # attached AFTER scheduling, mirrored onto their LDWEIGHTS
            # (the PE reorder window pulls LDWEIGHTS ahead of in-flight
            # matmuls; an in-context wait would deadlock the Tile
            # scheduling simulator, which only models the tile block).
            def p1mm(s, c, start, stop):
                if s == 0:
                    vh = kv0[:, DV : 2 * DV]
                    kh = kv0[:, c * P : (c + 1) * P]
                else:
                    base = (s - 1) * CHUNK
                    vh = kvt[:, base + DV : base + 2 * DV]
                    kh = kvt[:, base + c * P : base + (c + 1) * P]
                return nc.tensor.matmul(psums[c][:, :], kh, vh, start=start, stop=stop)

            kv_gates = []
            for s in range(SC - 2):
                for c in range(DC):
                    inst = p1mm(s, c, s == 0, False)
                    if s == 0:
                        kv_gates.append((inst, 16))
            # last two s-chunks c-major, so each psums[c] stops (and its
            # fp16 cast starts) several matmuls before phase-1 ends --
            # phase 2's first matmuls then aren't serialized on the casts
            for c in range(DC):
                p1mm(SC - 2, c, False, False)
                p1mm(SC - 1, c, False, True)

            # M PSUM -> SBUF fp16 casts.  c0/c2 on DVE, c1/c3 on ACT - all
            # four finish right as phase 2 needs them, and neither engine
            # delays its own phase-2 pipeline work.
            mhis = []
            for c in range(DC):
                mhi = mp_pool.tile([P, DV], f16, tag=f"mh{c}", name=f"mhi{c}")
                if c % 2 == 0:
                    nc.vector.tensor_copy(mhi[:, :], psums[c][:, :])
                else:
                    nc.scalar.copy(mhi[:, :], psums[c][:, :])
                mhis.append(mhi)

            # Phase 2: per pair of query tiles (shared width W):
            # X = Q M into PSUM, with the -60000 strip accumulated by an
            # extra identity-stationary matmul in the same group; negated
            # max-reduce from PSUM (DVE); exp with bias from PSUM
            # (ScalarE); one fp16 store per pair on the Sync ring.
            ident = qmt[:, 0:P]
            for i in range(NPAIR):
                W = pw[i]
                ex = expo.tile([P, 2 * DV], f16, tag="e")
                for h in range(2):
                    px = psum_x.tile([P, DV], f32, tag="x")
                    s = ss[2 * i + h]
                    wh = whs[2 * i + h]
                    qbase = HDR + (2 * i + h) * DC * P
                    for c in range(DC):
                        qh = qmt[:, qbase + c * P : qbase + (c + 1) * P]
                        nc.tensor.matmul(
                            px[:, 0:wh],
                            qh,
                            mhis[c][:, 0:wh],
                            start=(c == 0),
                            stop=(c == DC - 1 and s == 0),
                        )
                    if s:
                        mlo = P + soffs[2 * i + h]
                        nc.tensor.matmul(
                            px[:, wh - s : wh],
                            ident,
                            qmt[:, mlo : mlo + s],
                            start=False,
                            stop=True,
                        )
                    nmx = stats.tile([P, 1], f32, tag="nmx")
                    nc.vector.tensor_reduce(
                        out=nmx,
                        in_=px[:, 0:wh],
                        axis=mybir.AxisListType.X,
                        op=mybir.AluOpType.max,
                        negate=True,
                    )
                    if wh < W:
                        nc.gpsimd.memset(ex[:, h * W + wh : (h + 1) * W], 0)
                    nc.scalar.activation(
                        ex[:, h * W : h * W + wh],
                        px[:, 0:wh],
                        mybir.ActivationFunctionType.Exp,
                        bias=nmx[:, :],
                        scale=1.0,
                    )
                nc.sync.dma_start(out=o_d[i, :, :, 0:W], in_=ex[:, 0 : 2 * W])

    # Attach the kv-chunk completion waits now that scheduling is done.
    # The wait must ALSO go on the immediately preceding LDWEIGHTS: it
    # reads the chunk's K columns, and the PE reorder window can pull it
    # ahead of in-flight matmuls (bacc's move_matmul_waits_to_ldweights
    # deliberately leaves a lone matmul wait on the matmul).
    import bass_rust as _br

    for inst, val in kv_gates:
        inst.wait_op(kv_sem, val, "sem-ge")
    gate_map = {id(inst.ins): val for inst, val in kv_gates}
    for f in nc.m.functions:
        for b in f.blocks:
            insts = b.instructions
            for idx, bi in enumerate(insts):
                val = gate_map.get(id(bi))
                if val is None:
                    continue
                j = idx - 1
                while j >= 0 and not isinstance(insts[j], mybir.InstLdweights):
                    j -= 1
                if j >= 0:
                    _br.wait_op(insts[j], kv_sem, val, "sem-ge", True)

    nc.compile()
    return nc


def _get_nc(pw, ss, whs):
    key = (tuple(pw), tuple(ss), tuple(whs))
    if key not in _CACHE:
        _install_ntff_hook()
        _CACHE[key] = _build(*key)
    return _CACHE[key]


def kernel(K, V, Q, valid_len, _trace=False):
    from concourse.bass_utils import run_bass_kernel_spmd

    K = np.asarray(K, dtype=np.float32)
    V = np.asarray(V, dtype=np.float32)
    Q = np.asarray(Q, dtype=np.float32)
    vl = np.asarray(valid_len).astype(np.int64)

    # sort queries by valid_len (row permutation; exact for row-wise softmax)
    perm = np.argsort(vl, kind="stable")
    vls = vl[perm]
    widths = []
    for t in range(QT_TILES):
        w = int(vls[t * P : (t + 1) * P].max()) + 1
        widths.append(min(DV, -(-w // 32) * 32))
    # consumption order: widest first; pair consecutive, width = pair max
    order = sorted(range(QT_TILES), key=lambda i: widths[i], reverse=True)
    pw = tuple(widths[order[2 * i]] for i in range(NPAIR))

    # per-half compute widths wh = ceil32(max_vl+1) (tile's own width, <=
    # pair width; the [wh, W) tail of ex is memset to 0 on device) and
    # strip extents [lo, wh) with lo = floor32(min_vl+1)
    ss = []
    los = []
    whs = []
    for idx in range(QT_TILES):
        t = order[idx]
        Wp = pw[idx // 2]
        wh = min(Wp, widths[t])
        min_vl = int(vls[t * P])  # rows sorted ascending within tile
        lo = min(wh, ((min_vl + 1) // 32) * 32)
        ss.append(wh - lo)
        los.append(lo)
        whs.append(wh)
    ss = tuple(ss)
    whs = tuple(whs)
    sum_s = sum(ss)

    # K/V interleaved per s-chunk, partition-major fp16
    kv = np.empty((B, P, SC * 2 * DV), dtype=np.float16)
    k16 = K.astype(np.float16).reshape(B, SC, P, DV)
    v16 = V.astype(np.float16).reshape(B, SC, P, DV)
    kv.reshape(B, P, SC, 2, DV)[:, :, :, 0, :] = k16.transpose(0, 2, 1, 3)
    kv.reshape(B, P, SC, 2, DV)[:, :, :, 1, :] = v16.transpose(0, 2, 1, 3)

    # identity + strips + Q^T packed in consumption order
    scale = np.float32(1.0 / math.sqrt(D))
    qp = (Q[:, perm, :] * scale).astype(np.float16)  # [B, SQ, D]
    qt = qp.reshape(B, QT_TILES, P, DC, P).transpose(0, 4, 1, 3, 2)  # [B,p,t,c,r]
    col = np.arange(DV, dtype=np.int64)
    hdr = P + sum_s
    qm = np.empty((B, P, hdr + QT_TILES * DC * P), dtype=np.float16)
    qm[:, :, 0:P] = np.eye(P, dtype=np.float16)[None, :, :]
    off = P
    for idx in range(QT_TILES):
        t = order[idx]
        s = ss[idx]
        if s:
            lo = los[idx]
            tile_vl = vls[t * P : (t + 1) * P]  # [128]
            strip = np.where(
                col[None, lo : lo + s] > tile_vl[:, None],
                np.float16(NEG_FILL),
                np.float16(0.0),
            )  # [128 rows, s]
            qm[:, :, off : off + s] = strip[None, :, :]
            off += s
    for idx, t in enumerate(order):
        qm[:, :, hdr + idx * DC * P : hdr + (idx + 1) * DC * P] = qt[
            :, :, t, :, :
        ].reshape(B, P, DC * P)

    nc = _get_nc(pw, ss, whs)
    in_maps = [{"kv": kv[b], "qm": qm[b]} for b in range(N_CORES)]
    res = run_bass_kernel_spmd(
        nc, in_maps, core_ids=list(range(N_CORES)), trace=_trace
    )
    # o[i, p, h, w] = exp tile order[2i+h], sorted-row p; unwritten cols are 0
    out = np.empty((B, SQ, DV), dtype=np.float32)
    e_sorted = np.empty((SQ, DV), dtype=np.float32)
    for b in range(N_CORES):
        o = np.asarray(res.results[b]["o"]).astype(np.float32)
        for i in range(NPAIR):
            for h in range(2):
                t = order[2 * i + h]
                e_sorted[t * P : (t + 1) * P, :] = o[i, :, h, :]
        out[b, perm, :] = e_sorted / e_sorted.sum(axis=-1, keepdims=True)
    if _trace:
        kernel.last_result = res
    return out
